# revision 1
# baseline (speedup 1.0000x reference)
"""Trainium2 Bass kernel for nn_Dynamics (RealNVP-style diffeomorphism dynamics).

Math restructuring (validated vs jax reference to ~1e-6 rel):
  reference out = -inv(G) @ y with G = J@J, J = L5...L0 (per-sample coupling-layer
  Jacobians). Each L = [[I,0],[C,diag(e^s)]] in (cond,trans) block order, so
  inv(J) = inv(L0)...inv(L5) applies in closed form:
      u_trans <- (u_trans - C @ u_cond) * e^{-s}
  => out = -inv(J)(inv(J) y): two cheap back-substitution passes; no 8x8 inverse,
  no explicit Jacobian chain. The kernel's heavy work is, per layer:
    MM1: U^T = W'^T x_cond (K=4 matmul, feats on partitions)   [PE]
    frac = (U' + b') mod 1  (argument reduction, U' = U/2pi)   [DVE]
    cos/sin via ACT Sin(2pi*frac + {pi/2, 0})                  [ACT, the bottleneck]
    MM2: [s;t] = A~ @ cosU,  [Ds;Dt] = P~ @ sinU (K=1024)      [PE]
    transposes + per-sample 4-vector algebra                   [PE/DVE]
  with C = diag(x2*e^s) Ds + Dt and e^s from a small polynomial (|s| < 0.35).

Sharding: pure data parallelism, batch split over 8 cores; params replicated.
"""
import sys

import numpy as np

try:
    import concourse.bass as bass
except ImportError:
    sys.path.insert(0, "/opt/trn_rl_repo")
    import concourse.bass as bass
import concourse.mybir as mybir
import concourse.tile as tile
from concourse.bass_utils import run_bass_kernel_spmd

f32 = mybir.dt.float32

DIM = 8
HALF = 4
NS = 1024
ND = 6
B = 16384
NCORES = 8
BC = B // NCORES            # 2048 samples per core
NT = BC // 128              # 16 sample-tiles of 128
NBLK = 4                    # blocks per core
TPB = NT // NBLK            # 4 sample-tiles (512 samples) per block
SBLK = TPB * 128            # 512
NCHUNK = 16                 # feature chunks per layer (2 machines x 8)
GRP = 4                     # chunks per psum group
NGRP = NCHUNK // GRP

SCALE = float(np.sqrt(2.0 / NS))
INV2PI = float(1.0 / (2.0 * np.pi))
TWOPI = float(2.0 * np.pi)

# ---- const tile layout (free-dim f32 offsets in the [128, CW] const tile) ----
IDENT_OFF = 0
WT_OFF = IDENT_OFF + 128                      # [5p, 128] per (k,c): 6*16*128
ACAT_OFF = WT_OFF + ND * NCHUNK * 128         # [128, 8] per (k,c): 6*16*8
PCAT_OFF = ACAT_OFF + ND * NCHUNK * 8         # [128, 32] per (k,c): 6*16*32
PI2_OFF = PCAT_OFF + ND * NCHUNK * 32         # [128, 1] constant pi/2
CW = PI2_OFF + 1

MAGIC = float(1.5 * 2 ** 23)
# Measured: ACT Sin spline is accurate only on ~[-pi, pi], so both the sin and
# cos passes get their own wrapped fraction in [-0.5, 0.5] turns.
# cos wrap: fc = frac + 1/4 - (frac >= 1/4), i.e. (x+1/4) - round(x+1/4).
GP_WRAP_GROUPS = ()   # groups whose cos-wrap runs on gpsimd instead of DVE

# exp(s) polynomial for |s| <= 0.4 (actual |s|max ~ 0.23): degree-5 Chebyshev fit
_cheb = np.polynomial.chebyshev.Chebyshev.fit(
    np.linspace(-0.4, 0.4, 2001), np.exp(np.linspace(-0.4, 0.4, 2001)), 5)
_EXPC = [float(c) for c in _cheb.convert(kind=np.polynomial.Polynomial).coef]


def pack_consts(Ws, bs, As):
    """Pack all per-layer constants into one [128, CW] f32 array."""
    cst = np.zeros((128, CW), np.float32)
    cst[:, IDENT_OFF:IDENT_OFF + 128] = np.eye(128, dtype=np.float32)
    cst[:, PI2_OFF] = np.float32(np.pi / 2)
    Ws = np.asarray(Ws, np.float32)
    bs = np.asarray(bs, np.float32)
    As = np.asarray(As, np.float32)
    for k in range(ND):
        for c in range(NCHUNK):
            m, cc = divmod(c, 8)
            fsl = slice(cc * 128, (cc + 1) * 128)
            W = Ws[k, m][fsl]                       # [128, 4]
            A = As[k, m][:, fsl]                    # [4, 128]
            b = bs[k, m][fsl]                       # [128]
            # MM1 stationary: lhsT[h, p] = W[p, h] / 2pi; row 4 = b / 2pi
            o = WT_OFF + (k * NCHUNK + c) * 128
            cst[0:4, o:o + 128] = (W.T * INV2PI)
            cst[4, o:o + 128] = b * INV2PI
            # MM2 cos stationary: outputs [s(4), t(4)]
            o = ACAT_OFF + (k * NCHUNK + c) * 8
            cst[:, o + 4 * m:o + 4 * m + 4] = (A.T * SCALE)
            # MM2 sin stationary: outputs [Ds(16), Dt(16)], ij = i*4+j
            o = PCAT_OFF + (k * NCHUNK + c) * 32
            P = (-SCALE) * (A[:, None, :] * W.T[None, :, :])   # [4i, 4j, 128f]
            cst[:, o + 16 * m:o + 16 * m + 16] = P.reshape(16, 128).T
    return cst


def _bcast_mid(ap, count):
    """Insert a step-0 dim before the innermost free dim."""
    return bass.AP(tensor=ap.tensor, offset=ap.offset,
                   ap=[*ap.ap[:-1], [0, count], ap.ap[-1]])


def _bcast_inner(ap, count):
    """Append a step-0 innermost free dim."""
    return bass.AP(tensor=ap.tensor, offset=ap.offset,
                   ap=[*ap.ap, [0, count]])


_NOPCNT = [0]


def legalize_multi_waits(nc):
    """This toolchain's walrus accepts at most ONE sync-wait per TPB
    instruction (2 for EventSemaphore), but Tile's sem-assignment attaches
    many (e.g. the kernel-tail drain waits on every proc). Hoist excess waits
    onto wait-only NoOps inserted immediately before, on the same engine."""
    for fn in nc.m.functions:
        for blk in fn.blocks:
            out = []
            changed = False
            for inst in blk.instructions:
                si = inst.sync_info
                ow = list(si.on_wait) if (si is not None and si.on_wait) else []
                cap = 2 if type(inst).__name__ == "InstEventSemaphore" else 1
                if len(ow) > cap:
                    changed = True
                    for w in ow[:-cap]:
                        nop = mybir.InstNoOp(name=f"waitnop_{_NOPCNT[0]}")
                        _NOPCNT[0] += 1
                        nop.engine = inst.engine
                        nop.sync_info = mybir.SyncInfo(on_wait=[w], on_update=[])
                        out.append(nop)
                    inst.sync_info = mybir.SyncInfo(
                        on_wait=ow[-cap:],
                        on_update=list(si.on_update) if si.on_update else [])
                out.append(inst)
            if changed:
                blk.instructions = out
    return nc


def build_kernel(debug=False):
    nc = bass.Bass("TRN2", target_bir_lowering=False, debug=False)
    consts = nc.dram_tensor("consts", [128, CW], f32, kind="ExternalInput").ap()
    x_in = nc.dram_tensor("x_in", [BC, DIM], f32, kind="ExternalInput").ap()
    out = nc.dram_tensor("out", [BC, DIM], f32, kind="ExternalOutput").ap()
    DW = 2048 * 4 + 256 + 16 + 32 + 32 + 384 + 96 + 512
    dbg = (nc.dram_tensor("dbg", [128, DW], f32, kind="ExternalOutput").ap()
           if debug else None)

    Sin = mybir.ActivationFunctionType.Sin
    AOT = mybir.AluOpType

    with tile.TileContext(nc) as tc:
        import contextlib
        with contextlib.ExitStack() as ctx:
            singles = ctx.enter_context(tc.tile_pool(name="singles", bufs=1))
            fpool = ctx.enter_context(tc.tile_pool(name="frac", bufs=2))
            cpool = ctx.enter_context(tc.tile_pool(name="cosu", bufs=3))
            spool = ctx.enter_context(tc.tile_pool(name="sinu", bufs=3))
            xcpool = ctx.enter_context(tc.tile_pool(name="xc", bufs=2))
            xppool = ctx.enter_context(tc.tile_pool(name="xp", bufs=2))
            stpool = ctx.enter_context(tc.tile_pool(name="st", bufs=2))
            stTpool = ctx.enter_context(tc.tile_pool(name="stT", bufs=2))
            capool = ctx.enter_context(tc.tile_pool(name="call", bufs=2))
            smallp = ctx.enter_context(tc.tile_pool(name="small", bufs=3))
            scrp = ctx.enter_context(tc.tile_pool(name="scr", bufs=2))
            pu_pool = ctx.enter_context(tc.tile_pool(name="pu", bufs=1, space="PSUM"))
            pc_pool = ctx.enter_context(tc.tile_pool(name="pc", bufs=1, space="PSUM"))
            ps_pool = ctx.enter_context(tc.tile_pool(name="ps", bufs=1, space="PSUM"))
            ptr_pool = ctx.enter_context(tc.tile_pool(name="ptr", bufs=1, space="PSUM"))
            pyT_pool = ctx.enter_context(tc.tile_pool(name="pyT", bufs=1, space="PSUM"))

            cst = singles.tile([128, CW], f32)
            nc.sync.dma_start(out=cst, in_=consts)
            xp_all = singles.tile([128, NT, DIM], f32)
            nc.sync.dma_start(out=xp_all, in_=x_in.rearrange("(t p) d -> p t d", p=128))
            u_all = singles.tile([128, NT, DIM], f32)

            ident = cst[:, IDENT_OFF:IDENT_OFF + 128]

            # engine absorbers: each engine's first instruction depends on one proc only
            dve_scr = singles.tile([1, 1], f32)
            nc.vector.tensor_copy(out=dve_scr, in_=cst[0:1, 0:1])   # DVE <- q_consts
            pe_scr = pyT_pool.tile([4, TPB, 128], f32, tag="y2T")
            nc.tensor.matmul(pe_scr[:, 0, :], ident[:, 0:4], ident, start=True, stop=True)

            prev_sinU = None   # for per-layer DVE dummy (ACT tick absorb)
            prev_psc = None    # for per-layer ACT dummy (PE tick absorb)

            for blk in range(NBLK):
                tsl = slice(blk * TPB, (blk + 1) * TPB)
                xp = xppool.tile([128, TPB, DIM], f32)
                nc.vector.tensor_copy(out=xp, in_=xp_all[:, tsl, :])

                C_all = capool.tile([128, ND, TPB, 16], f32)
                ei_all = capool.tile([128, ND, TPB, HALF], f32)

                # initial conditioner transpose: x[:, 0:4] -> xc [4, 512]
                y2T = pyT_pool.tile([4, TPB, 128], f32, tag="y2T")
                for c in range(TPB):
                    nc.tensor.transpose(y2T[:, c, :], xp[:, c, 0:HALF], ident)
                xc = xcpool.tile([HALF + 1, SBLK], f32)
                nc.vector.memset(xc, 1.0)
                nc.vector.tensor_copy(out=xc[0:HALF, :], in_=y2T)

                for k in range(ND):
                    even = (k % 2 == 0)
                    cond_sl = slice(0, HALF) if even else slice(HALF, DIM)
                    trans_sl = slice(HALF, DIM) if even else slice(0, HALF)

                    # per-layer absorbers
                    if prev_sinU is not None:
                        scr = scrp.tile([1, 1], f32)
                        nc.vector.tensor_copy(out=scr, in_=prev_sinU[0:1, 0:1, 0:1])
                    if prev_psc is not None:
                        scr2 = scrp.tile([1, 1], f32)
                        nc.scalar.copy(out=scr2, in_=prev_psc[0:1, 0:1])

                    psc = pc_pool.tile([8, SBLK], f32)
                    pss = ps_pool.tile([32, SBLK], f32)
                    cos_tiles = []
                    sin_tiles = []
                    for g in range(NGRP):
                        pu = pu_pool.tile([128, GRP, 512], f32)
                        for c4 in range(GRP):
                            c = g * GRP + c4
                            wsl = cst[0:5, WT_OFF + (k * NCHUNK + c) * 128:
                                      WT_OFF + (k * NCHUNK + c) * 128 + 128]
                            nc.tensor.matmul(pu[:, c4, :], wsl, xc,
                                             start=True, stop=True)
                        # frac = x - round(x) in [-0.5,0.5], x = (W x1 + b)/2pi
                        kk = fpool.tile([128, GRP, 512], f32)
                        nc.vector.tensor_scalar(out=kk, in0=pu, scalar1=MAGIC,
                                                scalar2=MAGIC, op0=AOT.add,
                                                op1=AOT.subtract)
                        frac = fpool.tile([128, GRP, 512], f32)
                        nc.vector.tensor_sub(frac, pu, kk)
                        sinU = spool.tile([128, GRP, 512], f32)
                        nc.scalar.activation(out=sinU, in_=frac, func=Sin,
                                             scale=TWOPI)
                        # cos wrap: fc = frac - ((frac >= 1/4) - 1/4)
                        weng = nc.gpsimd if g in GP_WRAP_GROUPS else nc.vector
                        g2 = fpool.tile([128, GRP, 512], f32)
                        weng.tensor_scalar(out=g2, in0=frac, scalar1=0.25,
                                           scalar2=0.25, op0=AOT.is_ge,
                                           op1=AOT.subtract)
                        fc = fpool.tile([128, GRP, 512], f32)
                        weng.tensor_sub(fc, frac, g2)
                        # fc = (x+1/4) - round(x+1/4), so Sin(2pi*fc) == cos(2pi*x)
                        cosU = cpool.tile([128, GRP, 512], f32)
                        nc.scalar.activation(out=cosU, in_=fc, func=Sin,
                                             scale=TWOPI)
                        if debug and blk == 0 and k == 0 and g == 0:
                            dbt = singles.tile([128, 2048], f32, tag="dbt")
                            nc.vector.tensor_copy(out=dbt,
                                                  in_=pu.rearrange("p a b -> p (a b)"))
                            nc.sync.dma_start(out=dbg[:, 0:2048], in_=dbt)
                            nc.sync.dma_start(out=dbg[:, 2048:4096],
                                              in_=frac.rearrange("p a b -> p (a b)"))
                            nc.sync.dma_start(out=dbg[:, 4096:6144],
                                              in_=sinU.rearrange("p a b -> p (a b)"))
                            nc.sync.dma_start(out=dbg[:, 6144:8192],
                                              in_=cosU.rearrange("p a b -> p (a b)"))
                        for c4 in range(GRP):
                            c = g * GRP + c4
                            asl = cst[:, ACAT_OFF + (k * NCHUNK + c) * 8:
                                      ACAT_OFF + (k * NCHUNK + c) * 8 + 8]
                            nc.tensor.matmul(psc, asl, cosU[:, c4, :],
                                             start=(c == 0), stop=(c == NCHUNK - 1),
                                             skip_group_check=True)
                        for c4 in range(GRP):
                            c = g * GRP + c4
                            psl = cst[:, PCAT_OFF + (k * NCHUNK + c) * 32:
                                      PCAT_OFF + (k * NCHUNK + c) * 32 + 32]
                            nc.tensor.matmul(pss, psl, sinU[:, c4, :],
                                             start=(c == 0), stop=(c == NCHUNK - 1),
                                             skip_group_check=True)
                        cos_tiles.append(cosU)
                        sin_tiles.append(sinU)
                    prev_sinU = sin_tiles[-1]
                    prev_psc = psc

                    # post: transpose [s,t,-,Ds,Dt] into sample-major layout.
                    # partition starts must be 32-aligned, so Ds/Dt sit at 32:64.
                    st = stpool.tile([64, SBLK], f32)
                    nc.vector.tensor_copy(out=st[0:8, :], in_=psc)
                    nc.vector.tensor_copy(out=st[32:64, :], in_=pss)
                    stT_ps = ptr_pool.tile([128, TPB, 64], f32)
                    for c in range(TPB):
                        nc.tensor.transpose(stT_ps[:, c, :],
                                            st[:, c * 128:(c + 1) * 128],
                                            ident[0:64, 0:64])
                    stT = stTpool.tile([128, TPB, 64], f32)
                    nc.vector.tensor_copy(out=stT, in_=stT_ps)

                    sT = stT[:, :, 0:4]
                    tT = stT[:, :, 4:8]
                    DsT = stT[:, :, 32:48].rearrange("p c (i j) -> p c i j", j=4)
                    DtT = stT[:, :, 48:64].rearrange("p c (i j) -> p c i j", j=4)

                    # es = exp(s) via degree-5 poly (Estrin)
                    c0, c1, c2, c3, c4_, c5 = _EXPC
                    s2 = smallp.tile([128, TPB, HALF], f32)
                    nc.vector.tensor_mul(s2, sT, sT)
                    p01 = smallp.tile([128, TPB, HALF], f32)
                    nc.vector.tensor_scalar(out=p01, in0=sT, scalar1=c1, scalar2=c0,
                                            op0=AOT.mult, op1=AOT.add)
                    p23 = smallp.tile([128, TPB, HALF], f32)
                    nc.vector.tensor_scalar(out=p23, in0=sT, scalar1=c3, scalar2=c2,
                                            op0=AOT.mult, op1=AOT.add)
                    p45 = smallp.tile([128, TPB, HALF], f32)
                    nc.vector.tensor_scalar(out=p45, in0=sT, scalar1=c5, scalar2=c4_,
                                            op0=AOT.mult, op1=AOT.add)
                    t1 = smallp.tile([128, TPB, HALF], f32)
                    nc.vector.tensor_mul(t1, s2, p23)
                    q = smallp.tile([128, TPB, HALF], f32)
                    nc.vector.tensor_add(q, p01, t1)
                    s4 = smallp.tile([128, TPB, HALF], f32)
                    nc.vector.tensor_mul(s4, s2, s2)
                    t2 = smallp.tile([128, TPB, HALF], f32)
                    nc.vector.tensor_mul(t2, s4, p45)
                    es = smallp.tile([128, TPB, HALF], f32)
                    nc.vector.tensor_add(es, q, t2)

                    nc.vector.reciprocal(out=ei_all[:, k], in_=es)

                    x2es = smallp.tile([128, TPB, HALF], f32)
                    nc.vector.tensor_mul(x2es, xp[:, :, trans_sl], es)
                    # C = x2es (bcast over j) * Ds + Dt
                    Cv = C_all[:, k].rearrange("p c (i j) -> p c i j", j=4)
                    nc.vector.tensor_tensor(out=Cv, in0=DsT,
                                            in1=_bcast_inner(x2es, 4), op=AOT.mult)
                    nc.vector.tensor_tensor(out=Cv, in0=Cv, in1=DtT, op=AOT.add)
                    # y2 = x2es + t -> state update
                    nc.vector.tensor_tensor(out=xp[:, :, trans_sl], in0=x2es,
                                            in1=tT, op=AOT.add)
                    if debug and blk == 0 and k == 0:
                        dbt2 = singles.tile([128, 256 + 16 + 32], f32, tag="dbt2")
                        nc.vector.tensor_copy(
                            out=dbt2[:, 0:256],
                            in_=stT.rearrange("p a b -> p (a b)"))
                        nc.vector.tensor_copy(
                            out=dbt2[:, 256:272],
                            in_=es.rearrange("p a b -> p (a b)"))
                        nc.vector.tensor_copy(
                            out=dbt2[:, 272:304],
                            in_=xp.rearrange("p a b -> p (a b)"))
                        nc.sync.dma_start(out=dbg[:, 8192:8192 + 304], in_=dbt2)

                    if k < ND - 1:
                        y2T = pyT_pool.tile([4, TPB, 128], f32, tag="y2T")
                        for c in range(TPB):
                            nc.tensor.transpose(y2T[:, c, :], xp[:, c, trans_sl],
                                                ident)
                        xc = xcpool.tile([HALF + 1, SBLK], f32)
                        nc.vector.memset(xc, 1.0)
                        nc.vector.tensor_copy(out=xc[0:HALF, :], in_=y2T)

                if debug and blk == 0:
                    dbt3 = singles.tile([128, 32 + 384 + 96], f32, tag="dbt3")
                    nc.vector.tensor_copy(out=dbt3[:, 0:32],
                                          in_=xp.rearrange("p a b -> p (a b)"))
                    nc.vector.tensor_copy(
                        out=dbt3[:, 32:416],
                        in_=C_all.rearrange("p a b c -> p (a b c)"))
                    nc.vector.tensor_copy(
                        out=dbt3[:, 416:512],
                        in_=ei_all.rearrange("p a b c -> p (a b c)"))
                    nc.sync.dma_start(out=dbg[:, 8496:8496 + 512], in_=dbt3)

                # two back-substitution passes: u <- inv(J) u, twice
                for _ in range(2):
                    for k in range(ND - 1, -1, -1):
                        even = (k % 2 == 0)
                        cond_sl = slice(0, HALF) if even else slice(HALF, DIM)
                        trans_sl = slice(HALF, DIM) if even else slice(0, HALF)
                        Cv = C_all[:, k].rearrange("p c (i j) -> p c i j", j=4)
                        ucond = xp[:, :, cond_sl]
                        tmp = smallp.tile([128, TPB, HALF, HALF], f32)
                        nc.vector.tensor_tensor(out=tmp, in0=Cv,
                                                in1=_bcast_mid(ucond, 4),
                                                op=AOT.mult)
                        mv = smallp.tile([128, TPB, HALF], f32)
                        nc.vector.tensor_reduce(out=mv, in_=tmp,
                                                axis=mybir.AxisListType.X,
                                                op=AOT.add)
                        nc.vector.tensor_sub(xp[:, :, trans_sl],
                                             xp[:, :, trans_sl], mv)
                        nc.vector.tensor_mul(xp[:, :, trans_sl],
                                             xp[:, :, trans_sl], ei_all[:, k])

                nc.vector.tensor_scalar_mul(out=u_all[:, tsl, :], in0=xp,
                                            scalar1=-1.0)

            nc.sync.dma_start(out=out.rearrange("(t p) d -> p t d", p=128),
                              in_=u_all)
    legalize_multi_waits(nc)
    return nc


_NC_CACHE = None


def kernel(x, Ws, bs, As):
    global _NC_CACHE
    x = np.ascontiguousarray(np.asarray(x, np.float32))
    cst = pack_consts(Ws, bs, As)
    if _NC_CACHE is None:
        _NC_CACHE = build_kernel()
    nc = _NC_CACHE
    in_maps = [
        {"consts": cst, "x_in": np.ascontiguousarray(x[i * BC:(i + 1) * BC])}
        for i in range(NCORES)
    ]
    res = run_bass_kernel_spmd(nc, in_maps, core_ids=list(range(NCORES)))
    return np.concatenate([r["out"] for r in res.results], axis=0)


if __name__ == "__main__":
    rng = np.random.default_rng(0)
    x = rng.standard_normal((B, DIM)).astype(np.float32)
    Ws = rng.standard_normal((ND, 2, NS, HALF)).astype(np.float32)
    bs = (rng.random((ND, 2, NS)) * 2 * np.pi).astype(np.float32)
    As = (rng.standard_normal((ND, 2, HALF, NS)) * 0.05).astype(np.float32)
    got = kernel(x, Ws, bs, As)
    print("ran, out shape", got.shape)



# revision 5
# speedup vs baseline: 2.3738x; 2.3738x over previous
"""Trainium2 Bass kernel for nn_Dynamics (RealNVP-style diffeomorphism dynamics).

Math (validated vs jax reference):
  out = -inv(J)(inv(J) y) via two closed-form back-substitution passes
  (J is a product of coupling-layer Jacobians [[I,0],[C,diag(e^s)]]).

Per layer, per core (2048 samples, 2048 RFF features):
  MM1 (PE, fp32r): pu = (W x1 + b)/2pi, features on partitions   [1 cyc/row]
  kk  (DVE):       kk = round(pu) via magic-number trick -> bf16
  kksub (PE):      pu += (-I) @ kk  => frac in PSUM, in [-0.5, 0.5]
  ACT (from PSUM): sinU = Sin(2pi*frac) -> bf16 ;  z = Sin(pi*frac) -> bf16
  zsq (DVE/ACT):   zsq = z*z (bf16)    [cosU = 1 - 2 zsq folded into MM2]
  MM2 (PE, bf16):  [s;t] = cs_const + Acat2 @ zsq ;  [Ds;Dt] = P @ sinU
  then transpose to sample-major, es = exp(s) poly, C = diag(x2 es) Ds + Dt.

Sharding: pure data parallelism, batch split over 8 cores; params replicated.
"""
import sys

import numpy as np

try:
    import concourse.bass as bass
except ImportError:
    sys.path.insert(0, "/opt/trn_rl_repo")
    import concourse.bass as bass
import concourse.mybir as mybir
import concourse.tile as tile
from concourse.bass_utils import run_bass_kernel_spmd

f32 = mybir.dt.float32
f32r = mybir.dt.float32r
bf16 = mybir.dt.bfloat16
fp16 = mybir.dt.float16

DIM = 8
HALF = 4
NS = 1024
ND = 6
B = 16384
NCORES = 8
BC = B // NCORES            # 2048 samples per core
NT = BC // 128              # 16 sample-tiles of 128
NBLK = 4                    # blocks per core
TPB = NT // NBLK            # 4 sample-tiles (512 samples) per block
SBLK = TPB * 128            # 512
NCHUNK = 16                 # feature chunks per layer (2 machines x 8)
GRP = 2                     # chunks per psum group
NGRP = NCHUNK // GRP

SCALE = float(np.sqrt(2.0 / NS))
INV2PI = float(1.0 / (2.0 * np.pi))
TWOPI = float(2.0 * np.pi)
PI = float(np.pi)
MAGIC = float(1.5 * 2 ** 23)

# ---- bf16 const tile layout (free-dim offsets in [128, CB] bf16 tile) ----
NEGI_OFF = 0                                  # -I [128, 128]
ACAT_OFF = NEGI_OFF + 128                     # [128, 8] per (k,c)
PCAT_OFF = ACAT_OFF + ND * NCHUNK * 8         # [128, 32] per (k,c)
CS_OFF = PCAT_OFF + ND * NCHUNK * 32          # [1, 8] per k (partition 0)
CB = CS_OFF + ND * 8

# engine-balance knobs
ACT_SQ_GROUPS = ()        # groups whose zsq runs on ACT Square instead of DVE
ST_COPY_SCALAR = True     # psc/pss PSUM->SBUF copies on ScalarE
STT_COPY_SCALAR = False   # stT copy engine

# exp(s) polynomial for |s| <= 0.4: degree-5 Chebyshev fit
_cheb = np.polynomial.chebyshev.Chebyshev.fit(
    np.linspace(-0.4, 0.4, 2001), np.exp(np.linspace(-0.4, 0.4, 2001)), 5)
_EXPC = [float(c) for c in _cheb.convert(kind=np.polynomial.Polynomial).coef]


def pack_consts(Ws, bs, As):
    """Pack constants: (constw f32r [5,12288], constb fp16 [128,CB],
    ident f32 [128,128])."""
    import ml_dtypes
    Ws = np.asarray(Ws, np.float32)
    bs = np.asarray(bs, np.float32)
    As = np.asarray(As, np.float32)

    cw = np.zeros((5, ND * NCHUNK * 128), np.float32)
    cb = np.zeros((128, CB), np.float32)
    cb[:, NEGI_OFF:NEGI_OFF + 128] = -np.eye(128, dtype=np.float32)
    for k in range(ND):
        for c in range(NCHUNK):
            m, cc = divmod(c, 8)
            fsl = slice(cc * 128, (cc + 1) * 128)
            W = Ws[k, m][fsl]                       # [128, 4]
            A = As[k, m][:, fsl]                    # [4, 128]
            b = bs[k, m][fsl]                       # [128]
            o = (k * NCHUNK + c) * 128
            cw[0:4, o:o + 128] = (W.T * INV2PI)
            cw[4, o:o + 128] = b * INV2PI
            o = ACAT_OFF + (k * NCHUNK + c) * 8
            cb[:, o + 4 * m:o + 4 * m + 4] = (-2.0 * SCALE) * A.T
            o = PCAT_OFF + (k * NCHUNK + c) * 32
            P = (-SCALE) * (A[:, None, :] * W.T[None, :, :])   # [4i, 4j, 128f]
            cb[:, o + 16 * m:o + 16 * m + 16] = P.reshape(16, 128).T
        for m in range(2):
            o = CS_OFF + k * 8 + 4 * m
            cb[0, o:o + 4] = SCALE * As[k, m].sum(axis=1)
    cbb = cb.astype(np.float16)
    ident = np.eye(128, dtype=np.float32)
    return cw, cbb, ident


def _bcast_mid(ap, count):
    return bass.AP(tensor=ap.tensor, offset=ap.offset,
                   ap=[*ap.ap[:-1], [0, count], ap.ap[-1]])


def _bcast_inner(ap, count):
    return bass.AP(tensor=ap.tensor, offset=ap.offset,
                   ap=[*ap.ap, [0, count]])


_NOPCNT = [0]


def legalize_multi_waits(nc):
    """Hoist excess sync-waits onto wait-only NoOps (walrus allows 1/inst)."""
    for fn in nc.m.functions:
        for blk in fn.blocks:
            out = []
            changed = False
            for inst in blk.instructions:
                si = inst.sync_info
                ow = list(si.on_wait) if (si is not None and si.on_wait) else []
                cap = 2 if type(inst).__name__ == "InstEventSemaphore" else 1
                if len(ow) > cap:
                    changed = True
                    for w in ow[:-cap]:
                        nop = mybir.InstNoOp(name=f"waitnop_{_NOPCNT[0]}")
                        _NOPCNT[0] += 1
                        nop.engine = inst.engine
                        nop.sync_info = mybir.SyncInfo(on_wait=[w], on_update=[])
                        out.append(nop)
                    inst.sync_info = mybir.SyncInfo(
                        on_wait=ow[-cap:],
                        on_update=list(si.on_update) if si.on_update else [])
                out.append(inst)
            if changed:
                blk.instructions = out
    return nc


def build_kernel():
    nc = bass.Bass("TRN2", target_bir_lowering=False, debug=False)
    constw = nc.dram_tensor("constw", [5, ND * NCHUNK * 128], f32r,
                            kind="ExternalInput").ap()
    constb = nc.dram_tensor("constb", [128, CB], fp16,
                            kind="ExternalInput").ap()
    constf = nc.dram_tensor("constf", [128, 128], f32,
                            kind="ExternalInput").ap()
    x_in = nc.dram_tensor("x_in", [BC, DIM], f32, kind="ExternalInput").ap()
    out = nc.dram_tensor("out", [BC, DIM], f32, kind="ExternalOutput").ap()

    Sin = mybir.ActivationFunctionType.Sin
    Square = mybir.ActivationFunctionType.Square
    AOT = mybir.AluOpType

    with tile.TileContext(nc) as tc:
        import contextlib
        with contextlib.ExitStack() as ctx:
            singles = ctx.enter_context(tc.tile_pool(name="singles", bufs=1))
            xcpool = ctx.enter_context(tc.tile_pool(name="xc", bufs=1))
            kpool = ctx.enter_context(tc.tile_pool(name="kk", bufs=2))
            spool = ctx.enter_context(tc.tile_pool(name="sinu", bufs=3))
            zpool = ctx.enter_context(tc.tile_pool(name="zz", bufs=2))
            qpool = ctx.enter_context(tc.tile_pool(name="zsq", bufs=3))
            xppool = ctx.enter_context(tc.tile_pool(name="xp", bufs=2))
            stpool = ctx.enter_context(tc.tile_pool(name="st", bufs=2))
            stTpool = ctx.enter_context(tc.tile_pool(name="stT", bufs=2))
            capool = ctx.enter_context(tc.tile_pool(name="call", bufs=2))
            smallp = ctx.enter_context(tc.tile_pool(name="small", bufs=3))
            scrp = ctx.enter_context(tc.tile_pool(name="scr", bufs=2))
            pu_pool = ctx.enter_context(tc.tile_pool(name="pu", bufs=2, space="PSUM"))
            pc_pool = ctx.enter_context(tc.tile_pool(name="pc", bufs=1, space="PSUM"))
            ps_pool = ctx.enter_context(tc.tile_pool(name="ps", bufs=1, space="PSUM"))
            ptr_pool = ctx.enter_context(tc.tile_pool(name="ptr", bufs=1, space="PSUM"))
            pyT_pool = ctx.enter_context(tc.tile_pool(name="pyT", bufs=1, space="PSUM"))

            wst = singles.tile([5, ND * NCHUNK * 128], f32r)
            nc.sync.dma_start(out=wst, in_=constw)
            cstb = singles.tile([128, CB], fp16)
            nc.sync.dma_start(out=cstb, in_=constb)
            ident = singles.tile([128, 128], f32)
            nc.sync.dma_start(out=ident, in_=constf)
            xp_all = singles.tile([128, NT, DIM], f32)
            nc.sync.dma_start(out=xp_all, in_=x_in.rearrange("(t p) d -> p t d", p=128))
            u_all = singles.tile([128, NT, DIM], f32)

            negI = cstb[:, NEGI_OFF:NEGI_OFF + 128]
            ones1 = singles.tile([1, SBLK], fp16)
            nc.vector.memset(ones1, 1.0)

            # persistent xc tiles (f32r, ones row preset), 2-block pipeline
            ones5 = singles.tile([HALF + 1, SBLK], f32)
            nc.vector.memset(ones5, 1.0)
            xcs = {}
            for t in range(4):
                x_t = xcpool.tile([HALF + 1, SBLK], f32r, tag=f"xc{t}")
                nc.vector.tensor_copy(out=x_t, in_=ones5)
                xcs[t] = x_t

            # engine absorbers
            dve_scr = singles.tile([1, 1], f32)
            nc.vector.tensor_copy(out=dve_scr, in_=ident[0:1, 0:1])
            pe_scr = pyT_pool.tile([4, TPB, 128], f32, tag="y2T")
            nc.tensor.matmul(pe_scr[:, 0, :], ident[:, 0:4], ident,
                             start=True, stop=True)

            prev_sinU = None
            prev_psc = None

            def wsl(k, c):
                o = (k * NCHUNK + c) * 128
                return wst[:, o:o + 128]

            def acat(k, c):
                o = ACAT_OFF + (k * NCHUNK + c) * 8
                return cstb[:, o:o + 8]

            def pcat(k, c):
                o = PCAT_OFF + (k * NCHUNK + c) * 32
                return cstb[:, o:o + 32]

            for blk in range(NBLK):
                tsl = slice(blk * TPB, (blk + 1) * TPB)
                xp = xppool.tile([128, TPB, DIM], f32)
                nc.vector.tensor_copy(out=xp, in_=xp_all[:, tsl, :])

                C_all = capool.tile([128, ND, TPB, 16], f32)
                ei_all = capool.tile([128, ND, TPB, HALF], f32)

                y2T = pyT_pool.tile([4, TPB, 128], f32, tag="y2T")
                for c in range(TPB):
                    nc.tensor.transpose(y2T[:, c, :], xp[:, c, 0:HALF], ident)
                xc = xcs[(blk % 2) * 2]
                nc.vector.tensor_copy(out=xc[0:HALF, :], in_=y2T)

                for k in range(ND):
                    even = (k % 2 == 0)
                    trans_sl = slice(HALF, DIM) if even else slice(0, HALF)

                    if prev_sinU is not None:
                        scr = scrp.tile([1, 1], f32)
                        nc.vector.tensor_copy(out=scr,
                                              in_=prev_sinU[0:1, 0:1, 0:1])
                    if prev_psc is not None:
                        scr2 = scrp.tile([1, 1], f32)
                        nc.scalar.copy(out=scr2, in_=prev_psc[0:1, 0:1])

                    psc = pc_pool.tile([8, SBLK], f32)
                    pss = ps_pool.tile([32, SBLK], f32)
                    csl = cstb[0:1, CS_OFF + k * 8:CS_OFF + k * 8 + 8]
                    nc.tensor.matmul(psc, csl, ones1, start=True, stop=False,
                                     skip_group_check=True)
                    for g in range(NGRP):
                        pu = pu_pool.tile([128, GRP, 512], f32)
                        for c2 in range(GRP):
                            c = g * GRP + c2
                            nc.tensor.matmul(pu[:, c2, :], wsl(k, c), xc,
                                             start=True, stop=True)
                        kk = kpool.tile([128, GRP, 512], fp16)
                        nc.vector.tensor_scalar(out=kk, in0=pu, scalar1=MAGIC,
                                                scalar2=MAGIC, op0=AOT.add,
                                                op1=AOT.subtract)
                        for c2 in range(GRP):
                            nc.tensor.matmul(pu[:, c2, :], negI, kk[:, c2, :],
                                             start=False, stop=True,
                                             skip_group_check=True)
                        sinU = spool.tile([128, GRP, 512], fp16)
                        nc.scalar.activation(out=sinU, in_=pu, func=Sin,
                                             scale=TWOPI)
                        zz = zpool.tile([128, GRP, 512], fp16)
                        nc.scalar.activation(out=zz, in_=pu, func=Sin,
                                             scale=PI)
                        zsq = qpool.tile([128, GRP, 512], fp16)
                        if g in ACT_SQ_GROUPS:
                            nc.scalar.activation(out=zsq, in_=zz, func=Square)
                        else:
                            nc.vector.tensor_mul(zsq, zz, zz)
                        for c2 in range(GRP):
                            c = g * GRP + c2
                            nc.tensor.matmul(psc, acat(k, c), zsq[:, c2, :],
                                             start=False,
                                             stop=(c == NCHUNK - 1),
                                             skip_group_check=True)
                        for c2 in range(GRP):
                            c = g * GRP + c2
                            nc.tensor.matmul(pss, pcat(k, c), sinU[:, c2, :],
                                             start=(c == 0),
                                             stop=(c == NCHUNK - 1),
                                             skip_group_check=True)
                    prev_sinU = sinU
                    prev_psc = psc

                    # transpose [s,t,-,Ds,Dt] into sample-major layout
                    st = stpool.tile([64, SBLK], f32)
                    if ST_COPY_SCALAR:
                        nc.scalar.copy(out=st[0:8, :], in_=psc)
                        nc.scalar.copy(out=st[32:64, :], in_=pss)
                    else:
                        nc.vector.tensor_copy(out=st[0:8, :], in_=psc)
                        nc.vector.tensor_copy(out=st[32:64, :], in_=pss)
                    stT_ps = ptr_pool.tile([128, TPB, 64], f32)
                    for c in range(TPB):
                        nc.tensor.transpose(stT_ps[:, c, :],
                                            st[:, c * 128:(c + 1) * 128],
                                            ident[0:64, 0:64])
                    stT = stTpool.tile([128, TPB, 64], f32)
                    if STT_COPY_SCALAR:
                        nc.scalar.copy(out=stT, in_=stT_ps)
                    else:
                        nc.vector.tensor_copy(out=stT, in_=stT_ps)

                    sT = stT[:, :, 0:4]
                    tT = stT[:, :, 4:8]
                    DsT = stT[:, :, 32:48].rearrange("p c (i j) -> p c i j", j=4)
                    DtT = stT[:, :, 48:64].rearrange("p c (i j) -> p c i j", j=4)

                    # es = exp(s) via degree-5 poly (Estrin)
                    c0, c1, c2_, c3, c4_, c5 = _EXPC
                    s2 = smallp.tile([128, TPB, HALF], f32)
                    nc.vector.tensor_mul(s2, sT, sT)
                    p01 = smallp.tile([128, TPB, HALF], f32)
                    nc.vector.tensor_scalar(out=p01, in0=sT, scalar1=c1,
                                            scalar2=c0, op0=AOT.mult,
                                            op1=AOT.add)
                    p23 = smallp.tile([128, TPB, HALF], f32)
                    nc.vector.tensor_scalar(out=p23, in0=sT, scalar1=c3,
                                            scalar2=c2_, op0=AOT.mult,
                                            op1=AOT.add)
                    p45 = smallp.tile([128, TPB, HALF], f32)
                    nc.vector.tensor_scalar(out=p45, in0=sT, scalar1=c5,
                                            scalar2=c4_, op0=AOT.mult,
                                            op1=AOT.add)
                    t1 = smallp.tile([128, TPB, HALF], f32)
                    nc.vector.tensor_mul(t1, s2, p23)
                    q = smallp.tile([128, TPB, HALF], f32)
                    nc.vector.tensor_add(q, p01, t1)
                    s4 = smallp.tile([128, TPB, HALF], f32)
                    nc.vector.tensor_mul(s4, s2, s2)
                    t2 = smallp.tile([128, TPB, HALF], f32)
                    nc.vector.tensor_mul(t2, s4, p45)
                    es = smallp.tile([128, TPB, HALF], f32)
                    nc.vector.tensor_add(es, q, t2)

                    nc.vector.reciprocal(out=ei_all[:, k], in_=es)

                    x2es = smallp.tile([128, TPB, HALF], f32)
                    nc.vector.tensor_mul(x2es, xp[:, :, trans_sl], es)
                    Cv = C_all[:, k].rearrange("p c (i j) -> p c i j", j=4)
                    nc.vector.tensor_tensor(out=Cv, in0=DsT,
                                            in1=_bcast_inner(x2es, 4),
                                            op=AOT.mult)
                    nc.vector.tensor_tensor(out=Cv, in0=Cv, in1=DtT,
                                            op=AOT.add)
                    nc.vector.tensor_tensor(out=xp[:, :, trans_sl], in0=x2es,
                                            in1=tT, op=AOT.add)

                    if k < ND - 1:
                        y2T = pyT_pool.tile([4, TPB, 128], f32, tag="y2T")
                        for c in range(TPB):
                            nc.tensor.transpose(y2T[:, c, :], xp[:, c, trans_sl],
                                                ident)
                        xc = xcs[(blk % 2) * 2 + ((k + 1) % 2)]
                        nc.vector.tensor_copy(out=xc[0:HALF, :], in_=y2T)

                # two back-substitution passes: u <- inv(J) u, twice
                for _ in range(2):
                    for k in range(ND - 1, -1, -1):
                        even = (k % 2 == 0)
                        cond_sl = slice(0, HALF) if even else slice(HALF, DIM)
                        trans_sl = slice(HALF, DIM) if even else slice(0, HALF)
                        Cv = C_all[:, k].rearrange("p c (i j) -> p c i j", j=4)
                        ucond = xp[:, :, cond_sl]
                        tmp = smallp.tile([128, TPB, HALF, HALF], f32)
                        nc.vector.tensor_tensor(out=tmp, in0=Cv,
                                                in1=_bcast_mid(ucond, 4),
                                                op=AOT.mult)
                        mv = smallp.tile([128, TPB, HALF], f32)
                        nc.vector.tensor_reduce(out=mv, in_=tmp,
                                                axis=mybir.AxisListType.X,
                                                op=AOT.add)
                        nc.vector.tensor_sub(xp[:, :, trans_sl],
                                             xp[:, :, trans_sl], mv)
                        nc.vector.tensor_mul(xp[:, :, trans_sl],
                                             xp[:, :, trans_sl], ei_all[:, k])

                nc.vector.tensor_scalar_mul(out=u_all[:, tsl, :], in0=xp,
                                            scalar1=-1.0)

            nc.sync.dma_start(out=out.rearrange("(t p) d -> p t d", p=128),
                              in_=u_all)
    legalize_multi_waits(nc)
    return nc


_NC_CACHE = None


def kernel(x, Ws, bs, As):
    global _NC_CACHE
    x = np.ascontiguousarray(np.asarray(x, np.float32))
    cw, cbb, ident = pack_consts(Ws, bs, As)
    if _NC_CACHE is None:
        _NC_CACHE = build_kernel()
    nc = _NC_CACHE
    in_maps = [
        {"constw": cw, "constb": cbb, "constf": ident,
         "x_in": np.ascontiguousarray(x[i * BC:(i + 1) * BC])}
        for i in range(NCORES)
    ]
    res = run_bass_kernel_spmd(nc, in_maps, core_ids=list(range(NCORES)))
    return np.concatenate([r["out"] for r in res.results], axis=0)


if __name__ == "__main__":
    rng = np.random.default_rng(0)
    x = rng.standard_normal((B, DIM)).astype(np.float32)
    Ws = rng.standard_normal((ND, 2, NS, HALF)).astype(np.float32)
    bs = (rng.random((ND, 2, NS)) * 2 * np.pi).astype(np.float32)
    As = (rng.standard_normal((ND, 2, HALF, NS)) * 0.05).astype(np.float32)
    got = kernel(x, Ws, bs, As)
    print("ran, out shape", got.shape)
